# revision 9
# baseline (speedup 1.0000x reference)
"""Trainium2 Bass kernel for CycleBalanceLoss.

loss = ALPHA * mean_b |sum_l adj[b, argmax_l, argmax_{l+1}]|
     + (1-ALPHA) * mean_{b,l} (logsumexp(logits[b,l,:]) - logits[b,l,t[b,l]])

Sharding: pure data parallel over the batch dim B=64 across 8 cores
(8 batches per core). Each core:
  - streams its logits shard [8, 128, 1024] through SBUF,
  - computes per-position top-1 value+index (DVE max / max_index),
  - exp+row-sum in one ScalarE activation pass (accum_out) -> logsumexp,
  - gathers target logits and the argmax-path adjacency weights with
    indirect DMA (so the 256MB adj_matrix is never streamed - only
    127 elements per batch are touched),
  - reduces everything to a single scalar partial loss with two tiny
    PE matmuls.
Host sums the 8 per-core partial scalars (the gather/unshard step).
"""

import numpy as np

B, L, N = 64, 128, 1024
NCORES = 8
BPC = B // NCORES  # batches per core
ALPHA = 0.7

_CACHE = {}


def _build(loop_n=1):
    """Build the per-core Bass program.

    loop_n > 1 wraps the whole computation in a device-side loop - used only
    by the benchmark harness to measure per-iteration device time free of
    host/tunnel overhead. The graded path always uses loop_n=1.
    """
    import contextlib

    import concourse.bacc as bacc
    import concourse.tile as tile
    from concourse import bass, mybir

    f32 = mybir.dt.float32
    i32 = mybir.dt.int32
    u32 = mybir.dt.uint32
    AF = mybir.ActivationFunctionType
    Alu = mybir.AluOpType
    AX = mybir.AxisListType

    nc = bacc.Bacc(
        "TRN2",
        target_bir_lowering=False,
        debug=False,
        num_devices=NCORES,
    )

    logits = nc.dram_tensor("logits", [BPC, L, N], f32, kind="ExternalInput")
    tfidx = nc.dram_tensor("tfidx", [L, BPC], i32, kind="ExternalInput")
    adj = nc.dram_tensor("adj", [BPC * N * N, 1], f32, kind="ExternalInput")
    out = nc.dram_tensor("out", [1, 1], f32, kind="ExternalOutput")

    logits_ap = logits.ap()
    logits_flat = logits_ap.rearrange("b l n -> (b l n)")[:, None]

    with tile.TileContext(nc) as tc:
        with (
            tc.tile_pool(name="xp", bufs=3) as xp,
            tc.tile_pool(name="ep", bufs=2) as ep,
            tc.tile_pool(name="sp", bufs=3) as sp,
            tc.tile_pool(name="acc", bufs=1) as accp,
            tc.tile_pool(name="psum", bufs=1, space="PSUM") as pp,
            tc.For_i(0, loop_n, 1) if loop_n > 1 else contextlib.nullcontext(),
        ):
            ones = accp.tile([L, 1], f32)
            nc.vector.memset(ones[:], 1.0)

            # target flat indices (host precomputed) and target-logit gather
            TF = accp.tile([L, BPC], i32)
            nc.sync.dma_start(TF[:], tfidx.ap())
            # NOTE: multi-column offset tables wedge the HW DGE - one
            # indirect DMA per column ([P,1] offsets) is the proven shape.
            XT = accp.tile([L, BPC], f32)
            for b in range(BPC):
                nc.gpsimd.indirect_dma_start(
                    out=XT[:, b : b + 1],
                    out_offset=None,
                    in_=logits_flat,
                    in_offset=bass.IndirectOffsetOnAxis(ap=TF[:, b : b + 1], axis=0),
                )

            LSE = accp.tile([L, BPC], f32)
            W = accp.tile([L, BPC], f32)  # rows 0..L-2 hold path weights

            for b in range(BPC):
                X = xp.tile([L, N], f32, tag="X")
                nc.sync.dma_start(X[:], logits_ap[b])

                max8 = sp.tile([L, 8], f32, tag="max8")
                nc.vector.max(max8[:], X[:])
                idx8 = sp.tile([L, 8], u32, tag="idx8")
                nc.vector.max_index(idx8[:], max8[:], X[:])

                negmax = sp.tile([L, 1], f32, tag="negmax")
                nc.vector.tensor_scalar_mul(negmax[:], max8[:, 0:1], -1.0)
                E = ep.tile([L, N], f32, tag="E")
                s = sp.tile([L, 1], f32, tag="s")
                nc.scalar.activation(
                    E[:], X[:], AF.Exp, bias=negmax[:], scale=1.0, accum_out=s[:]
                )
                lns = sp.tile([L, 1], f32, tag="lns")
                nc.scalar.activation(lns[:], s[:], AF.Ln)
                nc.vector.tensor_add(LSE[:, b : b + 1], lns[:], max8[:, 0:1])

                # pair[l] = idx[l]*N + idx[l+1]; realign idx[l+1] onto
                # partitions 0..L-2 with a tiny SBUF->SBUF DMA first.
                dsh = sp.tile([L, 1], u32, tag="dsh")
                nc.sync.dma_start(dsh[0 : L - 1, :], idx8[1:L, 0:1])
                srcsh = sp.tile([L, 1], u32, tag="srcsh")
                nc.vector.tensor_scalar(
                    srcsh[0 : L - 1, :],
                    idx8[0 : L - 1, 0:1],
                    10,
                    None,
                    op0=Alu.logical_shift_left,
                )
                pair = sp.tile([L, 1], u32, tag="pair")
                nc.vector.tensor_tensor(
                    pair[0 : L - 1, :],
                    srcsh[0 : L - 1, :],
                    dsh[0 : L - 1, :],
                    op=Alu.add,
                )
                nc.gpsimd.indirect_dma_start(
                    out=W[0 : L - 1, b : b + 1],
                    out_offset=None,
                    in_=adj.ap(),
                    in_offset=bass.IndirectOffsetOnAxis(ap=pair[0 : L - 1, :], axis=0),
                    element_offset=b * N * N,
                )

            # cross-entropy: nll[l,b] = lse - x_target; scaled row-sum in R
            # col 0. (tensor_tensor_reduce would fuse this but it wedges the
            # device on this HW path - keep the three plain DVE ops.)
            R = accp.tile([L, 2], f32)
            nc.vector.memset(R[:, 1:2], 0.0)
            NLL = accp.tile([L, BPC], f32)
            nc.vector.tensor_sub(NLL[:], LSE[:], XT[:])
            nc.vector.reduce_sum(R[:, 0:1], NLL[:], axis=AX.X)
            nc.vector.tensor_scalar_mul(R[:, 0:1], R[:, 0:1], (1.0 - ALPHA) / (B * L))

            # balance: per-batch path sums via PE, ALPHA/B * |.| into R col 1
            ps_b = pp.tile([BPC, 1], f32)
            nc.tensor.matmul(
                out=ps_b[:],
                lhsT=W[0 : L - 1, :],
                rhs=ones[0 : L - 1, :],
                start=True,
                stop=True,
            )
            nc.scalar.activation(R[0:BPC, 1:2], ps_b[:], AF.Abs, scale=ALPHA / B)

            # column sums, then add the two partial sums -> scalar partial loss
            ps2 = pp.tile([2, 1], f32)
            nc.tensor.matmul(out=ps2[:], lhsT=R[:], rhs=ones[:], start=True, stop=True)
            c2 = sp.tile([2, 1], f32, tag="c2")
            nc.vector.tensor_copy(c2[:], ps2[:])
            ps1 = pp.tile([1, 1], f32)
            nc.tensor.matmul(
                out=ps1[:], lhsT=c2[:], rhs=ones[0:2, :], start=True, stop=True
            )
            otile = sp.tile([1, 1], f32, tag="otile")
            nc.vector.tensor_copy(otile[:], ps1[:])
            nc.sync.dma_start(out.ap(), otile[:])

    nc.compile()
    return nc


def _get_nc():
    if "nc" not in _CACHE:
        _CACHE["nc"] = _build()
    return _CACHE["nc"]


def make_in_maps(path_logits, target_paths, adj_matrix):
    """Shard full inputs into per-core in_maps (host-side packing only)."""
    l_off = np.arange(L, dtype=np.int64) * N  # [L]
    b_off = np.arange(BPC, dtype=np.int64)[:, None] * (L * N)  # [BPC, 1]
    in_maps = []
    for c in range(NCORES):
        sl = slice(c * BPC, (c + 1) * BPC)
        lg = np.ascontiguousarray(path_logits[sl], dtype=np.float32)
        ad = np.ascontiguousarray(adj_matrix[sl], dtype=np.float32).reshape(
            BPC * N * N, 1
        )
        t = np.asarray(target_paths[sl], dtype=np.int64)  # [BPC, L]
        tf = (b_off + l_off[None, :] + t).astype(np.int32)  # [BPC, L]
        tfT = np.ascontiguousarray(tf.T)  # [L, BPC]
        in_maps.append({"logits": lg, "tfidx": tfT, "adj": ad})
    return in_maps


def kernel(**inputs):
    from concourse import bass_utils

    nc = _get_nc()
    in_maps = make_in_maps(
        inputs["path_logits"], inputs["target_paths"], inputs["adj_matrix"]
    )
    res = bass_utils.run_bass_kernel_spmd(nc, in_maps, core_ids=list(range(NCORES)))
    total = np.float32(0.0)
    for r in res.results:
        total = total + np.float32(r["out"][0, 0])
    return np.asarray(total, dtype=np.float32)
